# revision 13
# baseline (speedup 1.0000x reference)
"""Cost-volume builder (correlation layer) for Trainium2, 8-core SPMD.

out[b, d, h, w] = (1/sqrt(C)) * sum_c feat1[b,c,h,w] * feat2[b,c,h+dy,w+dx]
for d = (dy+4)*9 + (dx+4), dy,dx in [-4,4]. B,C,H,W = 4,128,192,256.

Sharding: 8 cores = 4 batches x 2 H-halves (96 rows each, feat2 halo +-4).

Per-core algorithm (single 96-row pass):
  Pass 1 (PE): per r-block k=0..12 (8 feat2 rows) x w-tile: matmul
    lhsT=F2win[C,8x16=128] (stationary) vs rhs=F1[C,16hx8w=128] ->
    band[(j,we),(h,w)] = sum_c F2[c,r0+j,we]*F1[c,h,w] in PSUM. Edge
    r-blocks (k=0/k=12) clip rhs to the valid 8 h rows.
  Stage (DVE/ACT): one [128,<=1024] PSUM->SBUF fp16 cast copy per
    2-bank group of 8 w-tiles.
  Pass 2 (PE): 128 one-hot selection matrices Sel[128,81]; per (t,wl)
    two PSUM-accumulated matmuls (N=384 over r-block panes 0..11/1..12)
    gather each output position's 81 displacement values. Two wl per
    2-bank PSUM tile -> one [81,2x384] out-copy per wl-pair.
  Stores: fp16 [81,3072] chunk per t on the gpsimd SWDGE queue; host
    reassembles (tt,wl,k,w0) order and casts fp32.
"""

import math

import numpy as np

B, C, H, W = 4, 128, 192, 256
D = 81
NCORES = 8
OH = H // 2            # 96 output rows per core
NSLAB = 13             # f2 slabs per core, 8 rows each
WT = 32                # w-tiles per row (T=8)
T = 8
WE = 16                # feat2 w-window per tile
F2W = W + 8            # 264, zero-padded W
F2H = OH + 8           # 104 rows incl halo
NT = NSLAB * WT        # 416 stage tiles per class
SCALE = 1.0 / math.sqrt(C)


def _build_sel():
    """[128, 128*81] fp16 one-hot selection matrices, class c=(h_off*8+wl).
    Weight-column order of pass-1 lhsT is (kappa, j): row = kappa*8 + j."""
    sel = np.zeros((128, 128, 81), np.float16)
    for h_off in range(16):
        for wl in range(8):
            cls = h_off * 8 + wl
            for j in range(8):
                dy = j + 4 - h_off
                if -4 <= dy <= 4:
                    for dxh in range(9):  # dxh = dx + 4
                        row = (wl + dxh) * 8 + j
                        col = (dy + 4) * 9 + dxh
                        sel[row, cls, col] = 1.0
    return sel.reshape(128, 128 * 81)


def _emit(tc, f1, f2, selt, out):
    """Emit the Tile program. f1:[C,96*W] f16, f2:[C,13*F2W*8] f16,
    selt:[C,128*81] f16, out:[D,8*3072] f16 (DRAM APs)."""
    import concourse.bass as bass
    import concourse.mybir as mybir

    dt = mybir.dt
    nc = tc.nc
    MS = bass.MemorySpace

    with (
        tc.tile_pool(name="const", bufs=1) as cpool,
        tc.tile_pool(name="f1p", bufs=1) as f1p,
        tc.tile_pool(name="f2p", bufs=4) as f2p,
        tc.tile_pool(name="stgp", bufs=1) as stgp,
        tc.tile_pool(name="outp", bufs=2) as outp,
        tc.tile_pool(name="ps1", bufs=4, space=MS.PSUM) as ps1,
        tc.tile_pool(name="ps2", bufs=4, space=MS.PSUM) as ps2,
    ):
        # ---- persistent tiles ----
        f1buf = f1p.tile([128, OH * W], dt.float16)     # rows h in [0, 96)
        selb = cpool.tile([128, 128 * 81], dt.float16)

        # ---- load schedule: alternate the two HWDGE rings, criticality
        # order slab0, f1c0, slab1, f1c1, ...; sel halves mid-stream.
        slab_tiles = {}

        def load_slab(s, eng):
            tile_ = f2p.tile([128, F2W * 8], dt.float16, tag="f2s")
            eng.dma_start(tile_[:, :], f2[:, s * F2W * 8 : (s + 1) * F2W * 8])
            slab_tiles[s] = tile_

        # f1 row chunks: [0,8), [8,24), [24,40), ... [88,96)
        F1CH = [0, 8, 24, 40, 56, 72, 88, 96]

        def load_f1(c, eng):
            a, b = F1CH[c] * W, F1CH[c + 1] * W
            eng.dma_start(f1buf[:, a:b], f1[:, a:b])

        SY, SC = nc.sync, nc.scalar
        load_slab(0, SY)
        load_f1(0, SC)          # rows 0..8 (tiny, unblocks k=0 fast)
        load_slab(1, SC)
        load_f1(1, SY)
        load_slab(2, SY)
        load_f1(2, SC)
        load_slab(3, SC)
        load_f1(3, SC)
        load_slab(4, SY)
        load_f1(4, SC)
        load_slab(5, SC)
        load_f1(5, SY)
        load_slab(6, SY)
        load_f1(6, SC)
        load_slab(7, SC)
        load_slab(8, SY)
        load_slab(9, SC)
        load_slab(10, SY)
        load_slab(11, SC)
        load_slab(12, SY)
        nc.scalar.dma_start(selb[:, 0 : 64 * 81], selt[:, 0 : 64 * 81])
        nc.sync.dma_start(selb[:, 64 * 81 :], selt[:, 64 * 81 :])

        f1v = f1buf[:, :].rearrange("p (h x) -> p h x", h=OH)
        # stage layout: col = cls * 416 + (k*32 + w0), cls = h_off*8 + wl
        stg = stgp.tile([128, 128 * NT], dt.float16)
        stv2 = stg[:, :].rearrange("p (c t) -> p c t", c=128)
        eng = 0

        # ---- pass 1: band matmuls ----
        for k in range(NSLAB):
            f2s = slab_tiles[k]
            r0 = 8 * k - 8
            lo = max(r0, 0)          # clipped rhs rows [lo, hi)
            hi = min(r0 + 16, OH)
            n = (hi - lo) * T        # rhs cols per w-tile (128 or 64)
            c0 = (lo - r0) * 8       # first psum col's cls
            for g in range(8):       # groups of 4 w-tiles per PSUM bank
                pt = ps1.tile([128, 4 * n], dt.float32, tag="ps1")
                for u in range(4):
                    w0 = g * 4 + u
                    lhsT = f2s[:, 64 * w0 : 64 * w0 + 128]     # [128,128]
                    rhs = f1v[:, lo:hi, 8 * w0 : 8 * w0 + T]
                    nc.tensor.matmul(
                        pt[:, u * n : (u + 1) * n],
                        lhsT,
                        rhs,
                        start=True,
                        stop=True,
                    )
                # psum col = u*n + (cls - c0)  ->  stage (cls, t0+u)
                t0 = k * 32 + g * 4
                src = pt[:, :].rearrange("p (u c) -> p c u", u=4)
                dst = stv2[:, c0 : c0 + n, t0 : t0 + 4]
                if eng == 0:
                    nc.vector.tensor_copy(dst, src)
                else:
                    nc.scalar.copy(dst, src)
                eng ^= 1

        # ---- pass 2: selection matmuls + out copies + chunked stores ----
        for t in range(8, 16):
            outt = outp.tile([81, 8 * 384], dt.float16, tag="outt")
            for wl in range(8):
                clsA = t * 8 + wl
                clsB = (t - 8) * 8 + wl
                p2 = ps2.tile([128, 384], dt.float32, tag="ps2")
                po = p2[0:81, :]
                nc.tensor.matmul(
                    po,
                    selb[:, clsA * 81 : (clsA + 1) * 81],
                    stv2[:, clsA, 0:384],    # tiles k=0..11
                    start=True,
                    stop=False,
                )
                nc.tensor.matmul(
                    po,
                    selb[:, clsB * 81 : (clsB + 1) * 81],
                    stv2[:, clsB, 32:416],   # tiles k=1..12
                    start=False,
                    stop=True,
                )
                dst = outt[:, wl * 384 : (wl + 1) * 384]
                if eng == 0:
                    nc.vector.tensor_copy(dst, po)
                else:
                    nc.scalar.copy(dst, po)
                eng ^= 1
                if wl % 2 == 1:
                    # store chunk (tt, wl-pair); sync HWDGE ring is idle
                    # once input loads finish, and is much faster than SWDGE
                    chunk = (t - 8) * 3072 + (wl - 1) * 384
                    nc.sync.dma_start(
                        out[:, chunk : chunk + 768],
                        outt[:, (wl - 1) * 384 : (wl + 1) * 384],
                    )


def _build_nc():
    import concourse.mybir as mybir
    import concourse.tile as tile
    from concourse import bacc

    dt = mybir.dt
    nc = bacc.Bacc("TRN2", target_bir_lowering=False, debug=False)
    f1 = nc.dram_tensor("f1", [C, OH * W], dt.float16, kind="ExternalInput")
    f2 = nc.dram_tensor(
        "f2", [C, NSLAB * F2W * 8], dt.float16, kind="ExternalInput"
    )
    selt = nc.dram_tensor("sel", [C, 128 * 81], dt.float16, kind="ExternalInput")
    out = nc.dram_tensor("out", [D, 8 * 3072], dt.float16, kind="ExternalOutput")
    with tile.TileContext(nc) as tc:
        _emit(tc, f1[:, :], f2[:, :], selt[:, :], out[:, :])
    nc.finalize()
    return nc


def _shard_inputs(feat1, feat2):
    sel = _build_sel()
    in_maps = []
    for core in range(NCORES):
        b, half = core // 2, core % 2
        h0 = half * OH
        f1s = np.ascontiguousarray(
            (feat1[b, :, h0 : h0 + OH, :] * SCALE).astype(np.float16).reshape(C, OH * W)
        )
        f2pad = np.zeros((C, F2H, F2W), np.float16)
        lo, hi = h0 - 4, h0 + OH + 4
        slo, shi = max(lo, 0), min(hi, H)
        f2pad[:, slo - lo : shi - lo, 4 : 4 + W] = feat2[b, :, slo:shi, :].astype(
            np.float16
        )
        # slab s -> f2pad rows [8s, 8s+8), transposed to [C, w, r] so each
        # (16 w x 8 r) matmul weight window is contiguous.
        slabs = np.zeros((C, NSLAB, F2W, 8), np.float16)
        for s in range(NSLAB):
            slabs[:, s] = f2pad[:, 8 * s : 8 * s + 8, :].transpose(0, 2, 1)
        in_maps.append(
            {
                "f1": f1s,
                "f2": np.ascontiguousarray(slabs.reshape(C, -1)),
                "sel": sel,
            }
        )
    return in_maps


def _unshard(results):
    """results: per-core dicts with 'out' [81, 8*3072] f16 ->
    full [B, D, H, W] f32."""
    full = np.zeros((B, D, H, W), np.float32)
    for core in range(NCORES):
        b, half = core // 2, core % 2
        o = results[core]["out"].reshape(D, 8, 8, 12, 32)  # d,tt,wl,k,w0
        o = o.transpose(0, 3, 1, 4, 2).reshape(D, OH, W)   # d,(k tt),(w0 wl)
        full[b, :, half * OH : (half + 1) * OH, :] = o.astype(np.float32)
    return full


def kernel(feat1, feat2):
    feat1 = np.asarray(feat1, dtype=np.float32)
    feat2 = np.asarray(feat2, dtype=np.float32)
    from concourse.bass_utils import run_bass_kernel_spmd

    nc = _build_nc()
    in_maps = _shard_inputs(feat1, feat2)
    res = run_bass_kernel_spmd(nc, in_maps, list(range(NCORES)))
    return _unshard(res.results)
